# revision 21
# baseline (speedup 1.0000x reference)
"""Trainium2 Bass kernel for CAttention:
    k      = einsum('bcit,i->bct', x, alpha)
    scores = einsum('bct,ts,bds->bcd', k, Wc, k)
    att    = softmax(scores, axis=-1)
    out    = einsum('bci,bint->bcnt', att, x)

Sharding: data-parallel over batch B=64 across 8 NeuronCores (8 batches/core).

fp16 I/O version: x is cast to fp16 on the host (input DMA halves), the
channel-mix output is written as fp16 (output DMA halves), while the whole
score/softmax path accumulates in fp32.  Host-side numpy emulation of this
exact pipeline measures rel-err ~8.5e-3 against the fp32 reference.

Per-core layout (per batch b):
    X SBUF tile [128, 8192] fp16: partition p = j*8 + d  (j in [0,16) =
    n-chunk, d in [0,8) = channel), free q = n2*64 + t with n = j*128 + n2.

    k-path : PE-only.  alpha is folded into 16 accumulating matmuls: chunk
             g covers n2 in [8g, 8g+8); stationary AC_g[(j,d), (n2l,d')] =
             delta_{dd'} alpha[j,g,n2l], moving X[:, 512g:512g+512].
             Even/odd chunks run CONCURRENTLY in PE column groups 0/1
             (psum base partition 0/64 -> col_grp).  The diagonal blocks
             (n2l==n2f) hold k partials by n2 mod 8; 8 fold matmuls with
             the evacuated Y as the STATIONARY side produce kT[t,d]
             directly (no transpose, no extra copies).
    scores : V = Wc @ kT; scores = kT.T @ V (fp32).
    softmax: stable (DVE -max as exp bias); att = e * (1/sum) normalized
             on ACT, transposed/replicated on PE into the fp16 block-diag
             mix stationary.
    mix    : block-diag(att^T) [128,128] fp16 stationary, X fp16 moving,
             16 matmuls of 512 cols; evacuation alternates ACT/DVE in
             1024-wide slices (plain dtype-converting copies).

DMA queues (HWDGE rings get starved ~3x by SWDGE's fatter packets under
contention, and each queue alone caps well below the 358 GB/s HBM limit,
so all three are used, each direction split evenly):
    sync  (HWDGE) : constants, then input quarters 1-2 of each batch
    gpsimd(SWDGE) : input quarters 3-4, output slices 4-7
    scalar(HWDGE) : output slices 0-3

HAM note: the PE clock-gates to 1.2 GHz after ~3.4us idle.  Input DMAs
are quarter-split so the PE's X-wait fragments into sub-window chunklets,
and the WHOLE mix of batch b is deferred: its 16 matmuls are interleaved
one-by-one between the softmax-chain ops of batch b+1, with their bulk
PSUM evacuations emitted AFTER the chain's tiny ACT/DVE ops so the chain
is never queued behind a 1.2us evacuation slice.
"""

import sys

for _p in ("/opt/trn_rl_repo",):
    if _p not in sys.path:
        sys.path.insert(0, _p)

import numpy as np

B, C, N, T = 64, 8, 2048, 64
NCORES = 8
BS = B // NCORES          # batches per core
J = 16                    # n-chunks on partitions
N2 = N // J               # 128, n-extent in free dim
P = J * C                 # 128 partitions
F = N2 * T                # 8192 free elems
G = 16                    # k-path chunks (8 n2-values each)
NL = N2 // G              # 8, n2-local per chunk
QW = 512                  # matmul free width (one PSUM bank)
EV = 1024                 # evacuation slice width (two PSUM banks)
FQ = F // 4               # 2048, input DMA quarter (two on each queue)
OH = 4 * EV               # output DMA split: ACT ring slices 0-3, SWDGE 4-7

_PROGRAM_CACHE = {}


def _build_program():
    from contextlib import ExitStack

    import concourse.bacc as bacc
    from concourse import mybir, tile

    fp32 = mybir.dt.float32
    fp16 = mybir.dt.float16
    nc = bacc.Bacc("TRN2", target_bir_lowering=False, debug=False)

    xs = nc.dram_tensor("xs", [BS, C, N, T], fp16, kind="ExternalInput").ap()
    # aux16 packed: AC[0:1024] | idsel2[1024:1088] | rep16[1088:1216] (rows
    # 0-7) | mask[1216:1344]
    aux16 = nc.dram_tensor("aux16", [P, 1344], fp16, kind="ExternalInput").ap()
    # aux32 packed: wcT[0:64] (rows 0-63) | id8[64:72] (rows 0-7)
    aux32 = nc.dram_tensor("aux32", [P, 72], fp32, kind="ExternalInput").ap()
    out = nc.dram_tensor("out", [BS, C, N, T], fp16, kind="ExternalOutput").ap()

    Exp = mybir.ActivationFunctionType.Exp
    Copy = mybir.ActivationFunctionType.Copy
    AX = mybir.AxisListType.X
    MAX = mybir.AluOpType.max
    MULT = mybir.AluOpType.mult

    with tile.TileContext(nc) as tc, ExitStack() as ctx:
        cpool = ctx.enter_context(tc.tile_pool(name="const", bufs=1))
        xpool = ctx.enter_context(tc.tile_pool(name="x", bufs=4))
        opool = ctx.enter_context(tc.tile_pool(name="o", bufs=3))
        spool = ctx.enter_context(tc.tile_pool(name="small", bufs=2))
        bdpool = ctx.enter_context(tc.tile_pool(name="bd", bufs=2))
        mixp = ctx.enter_context(tc.tile_pool(name="mixp", bufs=3, space="PSUM"))
        psmall = ctx.enter_context(tc.tile_pool(name="psmall", bufs=2, space="PSUM"))

        a16_t = cpool.tile([P, 1344], fp16)
        nc.sync.dma_start(a16_t[:], aux16)
        a32_t = cpool.tile([P, 72], fp32)
        nc.sync.dma_start(a32_t[:], aux32)
        ac_t = a16_t[:, 0:1024]
        idsel_t = a16_t[:, 1024:1088]
        rep16_t = a16_t[:C, 1088:1216]
        mask_t = a16_t[:, 1216:1344]
        wcT_t = a32_t[:T, 0:64]
        id8_t = a32_t[:C, 64:72]

        def phase_in(b):
            # quarters: sems fire per quarter, so the PE's X-wait fragments
            # into sub-HAM-window chunklets and the clock stays at 2.4 GHz
            X = xpool.tile([P, F], fp16, tag="X")
            src = xs[b].rearrange("d (j n2) t -> j d (n2 t)", j=J)
            # batch 0 entirely on SWDGE: its fat packets win the queue
            # round-robin, landing the critical first tile ~5us earlier
            sync_qs = () if b == 0 else (0, 1)
            for q in range(4):
                eng = nc.sync if q in sync_qs else nc.gpsimd
                eng.dma_start(
                    X[:, q * FQ : (q + 1) * FQ], src[:, :, q * FQ : (q + 1) * FQ]
                )
            return X

        def phase_k(b, X):
            """PE k-path: 2x8 col-group-packed accumulating matmuls, then 8
            fold matmuls with Y stationary that land kT[t,d] directly."""
            yA = psmall.tile([P, QW], fp32, tag="ps")
            yB = psmall.tile([P, QW], fp32, tag="ps")
            for g in range(G):
                y = yA[0:T] if g % 2 == 0 else yB[T:P]
                nc.tensor.matmul(
                    y,
                    lhsT=ac_t[:, g * 64 : (g + 1) * 64],
                    rhs=X[:, g * QW : (g + 1) * QW],
                    start=(g < 2),
                    stop=(g >= G - 2),
                )
            y_sb = spool.tile([P, QW], fp16, tag="ysb")
            nc.scalar.copy(y_sb[0:T], yA[0:T])
            nc.vector.tensor_copy(y_sb[T:P], yB[T:P])
            kT_ps = psmall.tile([T, C], fp32, tag="ps")
            for l in range(NL):
                nc.tensor.matmul(
                    kT_ps[:],
                    lhsT=y_sb[:, l * T : (l + 1) * T],
                    rhs=idsel_t[:, l * C : (l + 1) * C],
                    start=(l == 0),
                    stop=(l == NL - 1),
                )
            kT_sb = spool.tile([T, C], fp32, tag="kTsb")
            nc.scalar.copy(kT_sb[:], kT_ps[:])
            return kT_sb

        def phase_chain(b, kT_sb, fillers):
            """Scores/softmax chain -> bd.  `fillers` are closures emitting
            one deferred mix matmul (b-1) each; a few are placed before every
            chain PE op so the PE never idles on a cross-engine wait, and
            their bulk evacuations are emitted after the chain's tiny ops."""
            fill = iter(fillers)

            def f(n=1):
                for _ in range(n):
                    nxt = next(fill, None)
                    if nxt is not None:
                        nxt()

            v_ps = psmall.tile([T, C], fp32, tag="ps")
            nc.tensor.matmul(v_ps[:], lhsT=wcT_t, rhs=kT_sb[:], start=True, stop=True)
            v_sb = spool.tile([T, C], fp32, tag="vsb")
            nc.scalar.copy(v_sb[:], v_ps[:])

            f(2)
            sc_ps = psmall.tile([C, C], fp32, tag="ps")
            nc.tensor.matmul(sc_ps[:], lhsT=kT_sb[:], rhs=v_sb[:], start=True, stop=True)

            negmax = spool.tile([C, 1], fp32, tag="negmax")
            nc.vector.tensor_reduce(negmax[:], sc_ps[:], axis=AX, op=MAX, negate=True)
            e_sb = spool.tile([C, C], fp32, tag="esb")
            ssum = spool.tile([C, 1], fp32, tag="ssum")
            nc.scalar.activation(e_sb[:], sc_ps[:], Exp, bias=negmax[:], accum_out=ssum[:])
            rcp = spool.tile([C, 1], fp32, tag="rcp")
            nc.vector.reciprocal(rcp[:], ssum[:])
            att_sb = spool.tile([C, C], fp32, tag="attsb")
            nc.scalar.activation(att_sb[:], e_sb[:], Copy, scale=rcp[:])

            f(3)
            aT_ps = psmall.tile([C, C], fp32, tag="ps")
            nc.tensor.transpose(aT_ps[:], att_sb[:], id8_t)
            aT_sb = spool.tile([C, C], fp16, tag="aTsb")
            nc.vector.tensor_copy(aT_sb[:], aT_ps[:])

            f(3)
            ar_ps = psmall.tile([P, C], fp32, tag="ps")
            nc.tensor.matmul(ar_ps[:], lhsT=rep16_t, rhs=aT_sb[:], start=True, stop=True)
            ar_sb = spool.tile([P, C], fp16, tag="arsb")
            nc.vector.tensor_copy(ar_sb[:], ar_ps[:])

            bd = bdpool.tile([P, P], fp16, tag="bd")
            nc.vector.tensor_tensor(
                out=bd[:].rearrange("p (j c) -> p j c", j=J),
                in0=mask_t.rearrange("p (j c) -> p j c", j=J),
                in1=ar_sb[:].rearrange("p (x c) -> p x c", x=1).to_broadcast([P, J, C]),
                op=MULT,
            )
            for nxt in fill:
                nxt()
            return bd

        def make_mix_fillers(b, X, bd):
            """Closures, one per mix matmul (16), with evacuation after each
            odd matmul and the two output DMAs after slices 3 and 7."""
            out_b = out[b].rearrange("c (j n2) t -> j c (n2 t)", j=J)
            ost_a = opool.tile([P, OH], fp16, tag="osta")
            ost_b = opool.tile([P, F - OH], fp16, tag="ostb")
            state = {}
            fillers = []
            for s in range(F // EV):
                for q in range(EV // QW):
                    def mk(s=s, q=q, b=b):
                        def emit():
                            if q == 0:
                                state["mp"] = mixp.tile(
                                    [P, EV], fp32, tag="mix", name="mp_mix"
                                )
                            mp = state["mp"]
                            base = s * EV + q * QW
                            nc.tensor.matmul(
                                mp[:, q * QW : (q + 1) * QW],
                                lhsT=bd[:],
                                rhs=X[:, base : base + QW],
                                start=True,
                                stop=True,
                            )
                            if q == EV // QW - 1:
                                if s < 4:
                                    dst = ost_a[:, s * EV : (s + 1) * EV]
                                else:
                                    dst = ost_b[:, (s - 4) * EV : (s - 3) * EV]
                                if s % 2 == 0:
                                    nc.scalar.copy(dst, mp[:])
                                else:
                                    nc.vector.tensor_copy(dst, mp[:])
                                if b == BS - 1 and s % 2 == 1:
                                    # last batch: drain in 2-slice pieces so
                                    # the final DMAs overlap the evacuation
                                    lo = (s - 1) * EV
                                    if s < 4:
                                        nc.scalar.dma_start(
                                            out_b[:, :, lo : lo + 2 * EV],
                                            ost_a[:, lo : lo + 2 * EV],
                                        )
                                    else:
                                        nc.gpsimd.dma_start(
                                            out_b[:, :, lo : lo + 2 * EV],
                                            ost_b[:, lo - OH : lo - OH + 2 * EV],
                                        )
                                elif b < BS - 1 and s == 3:
                                    nc.scalar.dma_start(out_b[:, :, :OH], ost_a[:])
                                elif b < BS - 1 and s == F // EV - 1:
                                    nc.gpsimd.dma_start(out_b[:, :, OH:], ost_b[:])
                        return emit
                    fillers.append(mk())
            return fillers

        # software-pipelined emission; the whole mix of batch b rides inside
        # chain(b+1) as PE fillers:
        #   PE: k(0) | chain(0) | k(1) | chain(1)+mix(0) | k(2) |
        #       chain(2)+mix(1) | ... | chain(7)+mix(6) | mix(7)
        X = [None] * BS
        X[0] = phase_in(0)
        kT_sb = phase_k(0, X[0])
        fillers = []
        for b in range(BS):
            bd = phase_chain(b, kT_sb, fillers)
            fillers = make_mix_fillers(b, X[b], bd)
            if b + 1 < BS:
                X[b + 1] = phase_in(b + 1)
                kT_sb = phase_k(b + 1, X[b + 1])
            X[b] = None
        for nxt in fillers:
            nxt()

    nc.compile()
    return nc


def _host_constants(Wc: np.ndarray, alpha: np.ndarray):
    # AC[(j*8+d), g*64 + n2l*8 + d'] = delta_{dd'} * alpha[j*128 + g*8 + n2l]
    a3 = np.asarray(alpha, dtype=np.float32).reshape(J, G, NL)
    ac = np.zeros((J, C, G, NL, C), dtype=np.float16)
    for d in range(C):
        ac[:, d, :, :, d] = a3
    ac = ac.reshape(P, G * 64)

    # idsel2[p, l*8+d'] = 1 if p mod 64 == l*8+d'  (sums both col-groups)
    idsel2 = np.tile(np.eye(T, dtype=np.float16), (2, 1))          # [128, 64]
    rep16 = np.tile(np.eye(C, dtype=np.float16), (1, J))           # [8, 128]
    mask = np.kron(
        np.eye(J, dtype=np.float16), np.ones((C, C), dtype=np.float16)
    )                                                              # [128, 128]
    aux16 = np.zeros((P, 1344), dtype=np.float16)
    aux16[:, 0:1024] = ac
    aux16[:, 1024:1088] = idsel2
    aux16[:C, 1088:1216] = rep16
    aux16[:, 1216:1344] = mask

    aux32 = np.zeros((P, 72), dtype=np.float32)
    aux32[:T, 0:64] = np.asarray(Wc, dtype=np.float32).T
    aux32[:C, 64:72] = np.eye(C, dtype=np.float32)
    return {
        "aux16": aux16,
        "aux32": aux32,
    }


def get_program():
    if "nc" not in _PROGRAM_CACHE:
        _PROGRAM_CACHE["nc"] = _build_program()
    return _PROGRAM_CACHE["nc"]


def run(x, Wc, alpha, trace=False, trace_kwargs=None):
    """Run on 8 cores; returns (full_output, BassKernelResults)."""
    from concourse.bass_utils import run_bass_kernel_spmd

    nc = get_program()
    consts = _host_constants(np.asarray(Wc), np.asarray(alpha))
    x16 = np.asarray(x, dtype=np.float16)
    in_maps = []
    for r in range(NCORES):
        m = {"xs": np.ascontiguousarray(x16[r * BS : (r + 1) * BS])}
        m.update(consts)
        in_maps.append(m)
    kw = {}
    if trace:
        kw["trace"] = True
        if trace_kwargs:
            kw.update(trace_kwargs)
    res = run_bass_kernel_spmd(nc, in_maps, list(range(NCORES)), **kw)
    out = np.concatenate([res.results[r]["out"] for r in range(NCORES)], axis=0)
    return out, res


def kernel(x, Wc, alpha):
    out, _ = run(x, Wc, alpha)
    return out.astype(np.float32)
